# revision 14
# baseline (speedup 1.0000x reference)
"""Causal self-attention (B=2, T=2048, C=1024, 16 heads) on 8 TRN2 NeuronCores.

Sharding: core = b*4 + hg  (b in {0,1} data-parallel over batch,
hg in {0..3} tensor-parallel over head groups of 4 heads).
Each core computes QKV projection for its 4 heads, causal attention, and a
partial output projection (its 256 rows of w_proj); the host sums the 4
partials per batch element and adds b_proj (the tensor-parallel all-reduce).

Device kernel design (per core):
- x / weights arrive as bf16 (halves HBM traffic; QKV+proj matmuls run
  bf16 at the same 1 cycle/row as fp32r, PSUM accumulates fp32).
- q,k produced transposed (channels x T) in fp32r; v natural (T x ch) with
  a ones-column FIRST per head so a single AV matmul also accumulates the
  softmax denominator at PSUM partition 0 (lhsT = [1 | 63 pad | v], M=128;
  channels land at PSUM partition 64 — wide PSUM APs must be 64-aligned).
- Scores computed transposed S^T (keys on partitions, queries free), exp on
  ScalarE; causal masking via restricted S/exp/AV column ranges + small
  triangular-mask multiplies on diagonal blocks. Diagonal blocks stream
  only their live column ranges (padded to >=256 so fp32r stays 1 cy/row).
- k stored zero-padded to 128 partitions per head (K=64 matmuls throttle
  the PE HAM clock gate; K=128 keeps the stream at 2.4GHz).
- Softmax normalization: denominator sits at PSUM partition 0, DVE
  reciprocal straight from PSUM, GpSimd partition_broadcast, one DVE
  multiply writing bf16 attn (the proj lhsT).
- Output projection epilogue: PSUM->SBUF copies on GpSimd (Pool engine,
  otherwise idle), y written bf16, one DMA per 128-row tile. b_proj is
  added on the host (exact: it commutes with the partial-sum gather).
- All inputs land in ~8 wide multi-dim DMAs instead of ~100 narrow ones:
  DMA issue on the sync queue costs ~600ns each and serialized issues were
  the old kernel's 21us dead startup.
"""
import numpy as np
from contextlib import ExitStack

import ml_dtypes

import concourse.bass as bass
import concourse.tile as tile
from concourse import bacc, mybir
from concourse.bass_utils import run_bass_kernel_spmd

F32 = mybir.dt.float32
F32R = mybir.dt.float32r
BF16 = mybir.dt.bfloat16
AF = mybir.ActivationFunctionType

B, T, C = 2, 2048, 1024
N_HEAD, HEAD_DIM = 16, 64
N_CORES = 8
H_LOC = 4          # heads per core
CQK = 512          # local q+k channels (4 heads * 64 * 2)
CV = 256           # local v channels
KT = 8             # contraction tiles over C (1024/128)
NTQ = 4            # T blocks of 512 (queries)
NT16 = 16          # T blocks of 128
SCALE = 1.0 / 8.0  # 1/sqrt(HEAD_DIM)

# consts layout (one packed [128, 646] f32 tensor)
CO_BQK = 0     # [128, 4]   qkv bias, one col per co block
CO_KM = 4      # [128, 2]   k zero-pad masks
CO_BV = 6      # [128, 256] v bias broadcast
CO_TRI = 262   # [128, 128] upper-triangular (incl diag) mask
CO_M2 = 390    # [128, 256] [zeros | triu] mask for m=3 diagonal blocks
NCONST = 646

_cached_nc = None


def _build():
    nc = bacc.Bacc("TRN2", target_bir_lowering=False, debug=False,
                   enable_asserts=False, num_devices=N_CORES)
    xt = nc.dram_tensor("xt", [C, T], BF16, kind="ExternalInput").ap()
    wqk = nc.dram_tensor("wqk", [C, CQK], BF16, kind="ExternalInput").ap()
    wv = nc.dram_tensor("wv", [C, CV], BF16, kind="ExternalInput").ap()
    wp = nc.dram_tensor("wp", [CV, C], BF16, kind="ExternalInput").ap()
    consts = nc.dram_tensor("consts", [128, NCONST], F32, kind="ExternalInput").ap()
    y = nc.dram_tensor("y", [T, C], BF16, kind="ExternalOutput").ap()

    with tile.TileContext(nc) as tc, ExitStack() as ctx:
        big = ctx.enter_context(tc.tile_pool(name="big", bufs=1))
        work = ctx.enter_context(tc.tile_pool(name="work", bufs=2))
        psum = ctx.enter_context(tc.tile_pool(name="psum", bufs=1, space="PSUM"))

        # ---- persistent SBUF tensors ----
        xt_sb = big.tile([128, KT * T], BF16, tag="xt")        # 32KB/p
        wqk_sb = big.tile([128, KT * CQK], BF16, tag="wqk")    # 8KB/p
        wv_sb = big.tile([128, KT * CV], BF16, tag="wv")       # 4KB/p
        wp_sb = big.tile([128, 2 * C], BF16, tag="wp")         # 4KB/p
        qk_sb = big.tile([128, 6 * T], F32R, tag="qk")         # 48KB/p
        # v_ext layout per (t16, head): 128 cols = [ones | 63 pad | v 64ch],
        # so av gets denom at PSUM partition 0 and channels at partition 64
        # (PSUM APs wider than 32 partitions must start 64-aligned).
        v_sb = big.tile([128, NT16 * (H_LOC * 128)], F32R, tag="v")  # 32KB/p
        attn_sb = big.tile([128, 2 * T], BF16, tag="attn")     # 8KB/p
        cn_sb = big.tile([128, NCONST], F32, tag="consts")

        bqk_sb = cn_sb[:, CO_BQK:CO_BQK + 4]
        kmask_sb = cn_sb[:, CO_KM:CO_KM + 2]
        bvbc_sb = cn_sb[:, CO_BV:CO_BV + CV]
        tri_sb = cn_sb[:, CO_TRI:CO_TRI + 128]
        mask2_sb = cn_sb[:, CO_M2:CO_M2 + 256]

        # ---- input DMAs: few wide multi-dim transfers, earliest-needed first
        # (SBUF side keeps the partition dim outermost; the DRAM side is
        # rearranged to iterate in the same p-major order.)
        # Transfers are serialized per DGE ring, so split across the two
        # HWDGE queues (SP=sync, Activation=scalar): sync takes the xt
        # chunk gating the first matmul, scalar takes weights + consts.
        QT = T // 4
        xt3_out = xt_sb[:].rearrange("p (k t) -> p k t", t=T)
        xt3_in = xt.rearrange("(k p) t -> p k t", p=128)

        nc.sync.dma_start(xt3_out[:, :, 0:QT], xt3_in[:, :, 0:QT])
        nc.scalar.dma_start(wqk_sb[:].rearrange("p (k c) -> p k c", c=CQK),
                            wqk.rearrange("(k p) c -> p k c", p=128))
        nc.scalar.dma_start(cn_sb[:], consts[:])
        nc.scalar.dma_start(wv_sb[:].rearrange("p (k c) -> p k c", c=CV),
                            wv.rearrange("(k p) c -> p k c", p=128))
        nc.sync.dma_start(xt3_out[:, :, QT:2 * QT], xt3_in[:, :, QT:2 * QT])
        nc.scalar.dma_start(xt3_out[:, :, 2 * QT:3 * QT], xt3_in[:, :, 2 * QT:3 * QT])
        nc.sync.dma_start(wp_sb[:].rearrange("p (k c) -> p k c", c=C),
                          wp.rearrange("(k p) c -> p k c", p=128))
        nc.sync.dma_start(xt3_out[:, :, 3 * QT:T], xt3_in[:, :, 3 * QT:T])

        # ---- QKV projection ----
        def qk_block(co, tq):
            # qk_t[co*128:(co+1)*128, tq*512:(tq+1)*512]
            p = psum.tile([128, 512], F32, tag="mm", bufs=2)
            for k in range(KT):
                nc.tensor.matmul(p[:],
                                 wqk_sb[:, k * CQK + co * 128: k * CQK + (co + 1) * 128],
                                 xt_sb[:, k * T + tq * 512: k * T + (tq + 1) * 512],
                                 start=(k == 0), stop=(k == KT - 1))
            if co < 2:
                nc.vector.tensor_scalar_add(qk_sb[:, co * T + tq * 512: co * T + (tq + 1) * 512],
                                            p[:], bqk_sb[:, co:co + 1])
            else:
                # k heads zero-padded to 128 partitions: kp tile for head h
                # holds k_h in its 64 rows, zeros elsewhere, so the S matmul
                # can contract K=128 (K=64 matmuls never unthrottle the PE).
                for half in range(2):
                    h = 2 * (co - 2) + half
                    nc.vector.tensor_scalar(
                        qk_sb[:, (2 + h) * T + tq * 512: (2 + h) * T + (tq + 1) * 512],
                        p[:], bqk_sb[:, co:co + 1], kmask_sb[:, half:half + 1],
                        mybir.AluOpType.add, mybir.AluOpType.mult)

        def v_block_pair(t16a, t16b):
            tiles = [t for t in (t16a, t16b) if t is not None]
            ps = []
            for t16 in tiles:
                p = psum.tile([128, CV], F32, tag="mm", bufs=2, name=f"vp{t16}")
                ps.append(p)
            for k in range(KT):
                for p, t16 in zip(ps, tiles):
                    nc.tensor.matmul(p[:],
                                     xt_sb[:, k * T + t16 * 128: k * T + (t16 + 1) * 128],
                                     wv_sb[:, k * CV:(k + 1) * CV],
                                     start=(k == 0), stop=(k == KT - 1))
            for p, t16 in zip(ps, tiles):
                out3 = v_sb[:, t16 * 512:(t16 + 1) * 512].rearrange("p (h d) -> p h d", d=128)[:, :, 64:128]
                in3 = p[:].rearrange("p (h d) -> p h d", d=64)
                b3 = bvbc_sb.rearrange("p (h d) -> p h d", d=64)
                nc.vector.tensor_add(out3, in3, b3)

        # ones columns of v_ext: one strided DVE write (in*0 + 1) — emitted
        # before any v write so the v tiles' other columns never wait on it.
        # The 31 pad columns are zeroed once on the Pool engine.
        ones_view = v_sb[:].rearrange("p (n d) -> p n d", d=128)[:, :, 0:1]
        nc.vector.tensor_scalar(ones_view, tri_sb[:, 0:64].rearrange("p (n d) -> p n d", d=1),
                                0.0, 1.0, mybir.AluOpType.mult, mybir.AluOpType.add)
        # zero the 63 pad columns (memset doesn't lower for f32r: write
        # src*0+0 instead, sourced from finite post-DMA xt data)
        nc.vector.tensor_scalar(
            v_sb[:].rearrange("p (n d) -> p n d", d=128)[:, :, 1:64],
            xt_sb[:, 0:64 * 63].rearrange("p (n d) -> p n d", d=63),
            0.0, 0.0, mybir.AluOpType.mult, mybir.AluOpType.add)

        def qkv_step(tq):
            qk_block(0, tq)
            qk_block(2, tq)
            v_block_pair(4 * tq, 4 * tq + 1)
            v_block_pair(4 * tq + 2, 4 * tq + 3)
            qk_block(1, tq)
            qk_block(3, tq)

        # ---- attention ----
        # Diagonal-block column starts: m = kt - 4*tqb >= 0 means keys
        # overlap queries. Live range is [128*m, 512); S/exp/AV stream
        # [c0, 512) with c0 = 128*m capped at 256 so fp32r keeps 1 cy/row.
        # m==3 therefore includes 128 dead columns which mask2 zeroes.
        def attn_head(h, tqb):
            co_q = h // 2
            kp = 2 + h
            p0 = 64 * (h % 2)
            nkt = 4 * (tqb + 1)
            av = psum.tile([128, 512], F32, tag="av", bufs=2)
            for g in range(nkt // 2):
                s = psum.tile([128, 1024], F32, tag="s", bufs=2)
                e = work.tile([128, 1024], F32R, tag="e", bufs=2)
                ms = [2 * g + j - 4 * tqb for j in range(2)]
                c0s = [min(max(m, 0) * 128, 256) for m in ms]
                for j in range(2):
                    kt = 2 * g + j
                    c0 = c0s[j]
                    nc.tensor.matmul(
                        s[:, j * 512 + c0: (j + 1) * 512],
                        qk_sb[:, kp * T + kt * 128: kp * T + (kt + 1) * 128],
                        qk_sb[:, co_q * T + tqb * 512 + c0: co_q * T + (tqb + 1) * 512],
                        start=True, stop=True)
                # exp via ScalarE (no max subtraction: |scores| <= ~8 here,
                # exp is safe in fp32). Restricted to live columns.
                if ms[1] < 1:
                    nc.scalar.activation(e[:], s[:], AF.Exp, scale=SCALE)
                elif ms[0] == 0:  # (m0, m1) pair: [0:512] and [128:512]
                    nc.scalar.activation(e[:, 0:512], s[:, 0:512], AF.Exp, scale=SCALE)
                    nc.scalar.activation(e[:, 512 + 128:1024], s[:, 512 + 128:1024],
                                         AF.Exp, scale=SCALE)
                else:  # (m2, m3) pair: both [256:512] — one strided act
                    ev = e[:].rearrange("p (j q) -> p j q", q=512)[:, :, 256:512]
                    sv = s[:].rearrange("p (j q) -> p j q", q=512)[:, :, 256:512]
                    nc.scalar.activation(ev, sv, AF.Exp, scale=SCALE)
                for j in range(2):
                    kt = 2 * g + j
                    m = ms[j]
                    c0 = c0s[j]
                    if m >= 0:  # diagonal block: triangular mask
                        if m == 3:
                            nc.vector.tensor_mul(e[:, j * 512 + 256: j * 512 + 512],
                                                 e[:, j * 512 + 256: j * 512 + 512],
                                                 mask2_sb)
                        else:
                            mc = 128 * m
                            nc.vector.tensor_mul(e[:, j * 512 + mc: j * 512 + mc + 128],
                                                 e[:, j * 512 + mc: j * 512 + mc + 128],
                                                 tri_sb)
                    nc.tensor.matmul(
                        av[:, c0:512],
                        v_sb[:, kt * 512 + h * 128: kt * 512 + (h + 1) * 128],
                        e[:, j * 512 + c0: (j + 1) * 512],
                        start=(kt == 0), stop=(kt == nkt - 1))
            # normalize: attn[:, cols] = av[64:128] * (1/av[0]).
            # av[0] is the denominator row at PSUM partition 0 (the custom
            # DVE reciprocal misreads nonzero partition offsets).
            recipf = work.tile([1, 512], F32, tag="recipf", bufs=1)
            nc.vector.reciprocal_approx_fast(recipf[:], av[0:1, :])
            bcs = work.tile([64, 512], F32, tag="bcs")
            nc.gpsimd.partition_broadcast(bcs[:], recipf[:])
            nc.vector.tensor_mul(
                attn_sb[p0:p0 + 64, (h // 2) * T + tqb * 512: (h // 2) * T + (tqb + 1) * 512],
                av[64:128, :], bcs[:])

        def proj_block(t16):
            ysb = work.tile([128, C], BF16, tag="y")
            for n in range(2):
                p = psum.tile([128, 512], F32, tag="mm", bufs=2)
                for kc in range(2):
                    nc.tensor.matmul(p[:],
                                     attn_sb[:, kc * T + t16 * 128: kc * T + (t16 + 1) * 128],
                                     wp_sb[:, kc * C + n * 512: kc * C + (n + 1) * 512],
                                     start=(kc == 0), stop=(kc == 1))
                # PSUM->SBUF drain split between ScalarE and DVE (GpSimd
                # cannot access PSUM); b_proj is folded in on the host.
                # Each 512-half DMAs out as soon as its copy lands.
                if n == 0:
                    nc.scalar.copy(ysb[:, n * 512:(n + 1) * 512], p[:])
                else:
                    nc.vector.tensor_copy(ysb[:, n * 512:(n + 1) * 512], p[:])
                nc.sync.dma_start(y[t16 * 128:(t16 + 1) * 128, n * 512:(n + 1) * 512],
                                  ysb[:, n * 512:(n + 1) * 512])

        # Software pipeline: QKV for tq+1 is spliced between attention heads
        # of tq so the PE has dense independent work while ScalarE runs exp.
        qkv_step(0)
        for tqb in range(NTQ):
            nxt = tqb + 1
            prv = tqb - 1
            attn_head(0, tqb)
            if nxt < NTQ:
                qk_block(0, nxt)
                qk_block(2, nxt)
            if prv >= 0:
                proj_block(4 * prv + 0)
                proj_block(4 * prv + 1)
            attn_head(1, tqb)
            if nxt < NTQ:
                v_block_pair(4 * nxt, 4 * nxt + 1)
                v_block_pair(4 * nxt + 2, 4 * nxt + 3)
            attn_head(2, tqb)
            if nxt < NTQ:
                qk_block(1, nxt)
            if prv >= 0:
                proj_block(4 * prv + 2)
            attn_head(3, tqb)
            if nxt < NTQ:
                qk_block(3, nxt)
            if prv >= 0:
                proj_block(4 * prv + 3)
        for t16 in range(4 * 3, 4 * 4):
            proj_block(t16)

    nc.compile()
    return nc


def _get_nc():
    global _cached_nc
    if _cached_nc is None:
        _cached_nc = _build()
    return _cached_nc


def make_in_maps(x, w_attn, b_attn, w_proj, b_proj):
    x = np.asarray(x, np.float32)
    w_attn = np.asarray(w_attn, np.float32)
    b_attn = np.asarray(b_attn, np.float32)
    w_proj = np.asarray(w_proj, np.float32)
    bf = ml_dtypes.bfloat16
    tri = np.triu(np.ones((128, 128), np.float32))
    mask2 = np.concatenate([np.zeros((128, 128), np.float32), tri], axis=1)
    kmask = np.concatenate([np.repeat([[1.0], [0.0]], 64, axis=0),
                            np.repeat([[0.0], [1.0]], 64, axis=0)],
                           axis=1).astype(np.float32)
    in_maps = []
    for core in range(N_CORES):
        b, hg = core // 4, core % 4
        cs = slice(hg * 256, (hg + 1) * 256)
        wqk_c = np.ascontiguousarray(
            np.concatenate([w_attn[:, cs], w_attn[:, 1024 + hg * 256:1024 + (hg + 1) * 256]],
                           axis=1)).astype(bf)
        bqk_vec = np.concatenate([b_attn[cs], b_attn[1024 + hg * 256:1024 + (hg + 1) * 256]])
        consts = np.zeros((128, NCONST), np.float32)
        consts[:, CO_BQK:CO_BQK + 4] = bqk_vec.reshape(4, 128).T
        consts[:, CO_KM:CO_KM + 2] = kmask
        consts[:, CO_BV:CO_BV + CV] = np.broadcast_to(
            b_attn[2048 + hg * 256:2048 + (hg + 1) * 256], (128, 256))
        consts[:, CO_TRI:CO_TRI + 128] = tri
        consts[:, CO_M2:CO_M2 + 256] = mask2
        in_maps.append({
            "xt": np.ascontiguousarray(x[b].T).astype(bf),
            "wqk": wqk_c,
            "wv": np.ascontiguousarray(w_attn[:, 2048 + hg * 256:2048 + (hg + 1) * 256]).astype(bf),
            "wp": np.ascontiguousarray(w_proj[cs, :]).astype(bf),
            "consts": consts,
        })
    return in_maps


def kernel(x, w_attn, b_attn, w_proj, b_proj):
    in_maps = make_in_maps(x, w_attn, b_attn, w_proj, b_proj)
    nc = _get_nc()
    res = run_bass_kernel_spmd(nc, in_maps, core_ids=list(range(N_CORES)))
    b_proj = np.asarray(b_proj, np.float32)
    y = np.zeros((B, T, C), np.float32)
    for core in range(N_CORES):
        y[core // 4] += np.asarray(res.results[core]["y"], dtype=np.float32)
    y += b_proj
    return y


# revision 15
# speedup vs baseline: 1.0333x; 1.0333x over previous
"""Causal self-attention (B=2, T=2048, C=1024, 16 heads) on 8 TRN2 NeuronCores.

Sharding: core = b*4 + hg  (b in {0,1} data-parallel over batch,
hg in {0..3} tensor-parallel over head groups of 4 heads).
Each core computes QKV projection for its 4 heads, causal attention, and a
partial output projection (its 256 rows of w_proj); the host sums the 4
partials per batch element and adds b_proj (the tensor-parallel all-reduce).

Device kernel design (per core):
- x / weights arrive as bf16 (halves HBM traffic; QKV+proj matmuls run
  bf16 at the same 1 cycle/row as fp32r, PSUM accumulates fp32).
- q,k produced transposed (channels x T) in fp32r; v natural (T x ch) with
  a ones-column FIRST per head so a single AV matmul also accumulates the
  softmax denominator at PSUM partition 0 (lhsT = [1 | 63 pad | v], M=128;
  channels land at PSUM partition 64 — wide PSUM APs must be 64-aligned).
- Scores computed transposed S^T (keys on partitions, queries free), exp on
  ScalarE; causal masking via restricted S/exp/AV column ranges + small
  triangular-mask multiplies on diagonal blocks. Diagonal blocks stream
  only their live column ranges (padded to >=256 so fp32r stays 1 cy/row).
- k stored zero-padded to 128 partitions per head (K=64 matmuls throttle
  the PE HAM clock gate; K=128 keeps the stream at 2.4GHz).
- Softmax normalization: denominator sits at PSUM partition 0, DVE
  reciprocal straight from PSUM, GpSimd partition_broadcast, one DVE
  multiply writing bf16 attn (the proj lhsT).
- Output projection epilogue: PSUM->SBUF copies on GpSimd (Pool engine,
  otherwise idle), y written bf16, one DMA per 128-row tile. b_proj is
  added on the host (exact: it commutes with the partial-sum gather).
- All inputs land in ~8 wide multi-dim DMAs instead of ~100 narrow ones:
  DMA issue on the sync queue costs ~600ns each and serialized issues were
  the old kernel's 21us dead startup.
"""
import numpy as np
from contextlib import ExitStack

import ml_dtypes

import concourse.bass as bass
import concourse.tile as tile
from concourse import bacc, mybir
from concourse.bass_utils import run_bass_kernel_spmd

F32 = mybir.dt.float32
F32R = mybir.dt.float32r
BF16 = mybir.dt.bfloat16
AF = mybir.ActivationFunctionType

B, T, C = 2, 2048, 1024
N_HEAD, HEAD_DIM = 16, 64
N_CORES = 8
H_LOC = 4          # heads per core
CQK = 512          # local q+k channels (4 heads * 64 * 2)
CV = 256           # local v channels
KT = 8             # contraction tiles over C (1024/128)
NTQ = 4            # T blocks of 512 (queries)
NT16 = 16          # T blocks of 128
SCALE = 1.0 / 8.0  # 1/sqrt(HEAD_DIM)

# consts layout (one packed [128, 646] f32 tensor)
CO_BQK = 0     # [128, 4]   qkv bias, one col per co block
CO_KM = 4      # [128, 2]   k zero-pad masks
CO_BV = 6      # [128, 256] v bias broadcast
CO_TRI = 262   # [128, 128] upper-triangular (incl diag) mask
CO_M2 = 390    # [128, 256] [zeros | triu] mask for m=3 diagonal blocks
NCONST = 646

_cached_nc = None


def _build():
    nc = bacc.Bacc("TRN2", target_bir_lowering=False, debug=False,
                   enable_asserts=False, num_devices=N_CORES)
    # All inputs arrive pre-relayouted host-side to match their SBUF tile
    # layout exactly, so every load is a plain 2D contiguous DMA (max-size
    # descriptors: [p, k, 512]-gather layouts generate 1024 x 1KB
    # descriptors and run at ~90GB/s; contiguous runs at full rate).
    xt = nc.dram_tensor("xt", [128, KT * T], BF16, kind="ExternalInput").ap()
    wqk = nc.dram_tensor("wqk", [128, KT * CQK], BF16, kind="ExternalInput").ap()
    wv = nc.dram_tensor("wv", [128, KT * CV], BF16, kind="ExternalInput").ap()
    wp = nc.dram_tensor("wp", [128, 2 * C], BF16, kind="ExternalInput").ap()
    consts = nc.dram_tensor("consts", [128, NCONST], F32, kind="ExternalInput").ap()
    y = nc.dram_tensor("y", [T, C], BF16, kind="ExternalOutput").ap()

    with tile.TileContext(nc) as tc, ExitStack() as ctx:
        big = ctx.enter_context(tc.tile_pool(name="big", bufs=1))
        work = ctx.enter_context(tc.tile_pool(name="work", bufs=2))
        psum = ctx.enter_context(tc.tile_pool(name="psum", bufs=1, space="PSUM"))

        # ---- persistent SBUF tensors ----
        xt_sb = big.tile([128, KT * T], BF16, tag="xt")        # 32KB/p
        wqk_sb = big.tile([128, KT * CQK], BF16, tag="wqk")    # 8KB/p
        wv_sb = big.tile([128, KT * CV], BF16, tag="wv")       # 4KB/p
        wp_sb = big.tile([128, 2 * C], BF16, tag="wp")         # 4KB/p
        qk_sb = big.tile([128, 6 * T], F32R, tag="qk")         # 48KB/p
        # v_ext layout per (t16, head): 128 cols = [ones | 63 pad | v 64ch],
        # so av gets denom at PSUM partition 0 and channels at partition 64
        # (PSUM APs wider than 32 partitions must start 64-aligned).
        v_sb = big.tile([128, NT16 * (H_LOC * 128)], F32R, tag="v")  # 32KB/p
        attn_sb = big.tile([128, 2 * T], BF16, tag="attn")     # 8KB/p
        cn_sb = big.tile([128, NCONST], F32, tag="consts")

        bqk_sb = cn_sb[:, CO_BQK:CO_BQK + 4]
        kmask_sb = cn_sb[:, CO_KM:CO_KM + 2]
        bvbc_sb = cn_sb[:, CO_BV:CO_BV + CV]
        tri_sb = cn_sb[:, CO_TRI:CO_TRI + 128]
        mask2_sb = cn_sb[:, CO_M2:CO_M2 + 256]

        # ---- input DMAs: plain 2D contiguous transfers, split across the
        # two HWDGE queues (SP=sync, Activation=scalar): sync takes the xt
        # chunk gating the first matmul, scalar takes weights + consts.
        # xt chunk q covers columns [q*4096, (q+1)*4096) = the tq block q
        # (chunk-major layout: col = tq*4096 + k*512 + t).
        CH = KT * 512  # 4096 cols per tq chunk

        nc.sync.dma_start(xt_sb[:, 0:CH], xt[:, 0:CH])
        nc.scalar.dma_start(wqk_sb[:], wqk[:])
        nc.scalar.dma_start(cn_sb[:], consts[:])
        nc.scalar.dma_start(wv_sb[:], wv[:])
        nc.sync.dma_start(xt_sb[:, CH:2 * CH], xt[:, CH:2 * CH])
        nc.scalar.dma_start(xt_sb[:, 2 * CH:3 * CH], xt[:, 2 * CH:3 * CH])
        nc.sync.dma_start(wp_sb[:], wp[:])
        nc.sync.dma_start(xt_sb[:, 3 * CH:4 * CH], xt[:, 3 * CH:4 * CH])

        # ---- QKV projection ----
        def qk_block(co, tq):
            # qk_t[co*128:(co+1)*128, tq*512:(tq+1)*512]
            p = psum.tile([128, 512], F32, tag="mm", bufs=2)
            for k in range(KT):
                nc.tensor.matmul(p[:],
                                 wqk_sb[:, k * CQK + co * 128: k * CQK + (co + 1) * 128],
                                 xt_sb[:, tq * 4096 + k * 512: tq * 4096 + (k + 1) * 512],
                                 start=(k == 0), stop=(k == KT - 1))
            if co < 2:
                nc.vector.tensor_scalar_add(qk_sb[:, co * T + tq * 512: co * T + (tq + 1) * 512],
                                            p[:], bqk_sb[:, co:co + 1])
            else:
                # k heads zero-padded to 128 partitions: kp tile for head h
                # holds k_h in its 64 rows, zeros elsewhere, so the S matmul
                # can contract K=128 (K=64 matmuls never unthrottle the PE).
                for half in range(2):
                    h = 2 * (co - 2) + half
                    nc.vector.tensor_scalar(
                        qk_sb[:, (2 + h) * T + tq * 512: (2 + h) * T + (tq + 1) * 512],
                        p[:], bqk_sb[:, co:co + 1], kmask_sb[:, half:half + 1],
                        mybir.AluOpType.add, mybir.AluOpType.mult)

        def v_block_pair(t16a, t16b):
            tiles = [t for t in (t16a, t16b) if t is not None]
            ps = []
            for t16 in tiles:
                p = psum.tile([128, CV], F32, tag="mm", bufs=2, name=f"vp{t16}")
                ps.append(p)
            for k in range(KT):
                for p, t16 in zip(ps, tiles):
                    nc.tensor.matmul(p[:],
                                     xt_sb[:, (t16 // 4) * 4096 + k * 512 + (t16 % 4) * 128:
                                            (t16 // 4) * 4096 + k * 512 + (t16 % 4 + 1) * 128],
                                     wv_sb[:, k * CV:(k + 1) * CV],
                                     start=(k == 0), stop=(k == KT - 1))
            for p, t16 in zip(ps, tiles):
                out3 = v_sb[:, t16 * 512:(t16 + 1) * 512].rearrange("p (h d) -> p h d", d=128)[:, :, 64:128]
                in3 = p[:].rearrange("p (h d) -> p h d", d=64)
                b3 = bvbc_sb.rearrange("p (h d) -> p h d", d=64)
                nc.vector.tensor_add(out3, in3, b3)

        # ones columns of v_ext: one strided DVE write (in*0 + 1) — emitted
        # before any v write so the v tiles' other columns never wait on it.
        # The 31 pad columns are zeroed once on the Pool engine.
        ones_view = v_sb[:].rearrange("p (n d) -> p n d", d=128)[:, :, 0:1]
        nc.vector.tensor_scalar(ones_view, tri_sb[:, 0:64].rearrange("p (n d) -> p n d", d=1),
                                0.0, 1.0, mybir.AluOpType.mult, mybir.AluOpType.add)
        # zero the 63 pad columns (memset doesn't lower for f32r: write
        # src*0+0 instead, sourced from finite post-DMA xt data)
        nc.vector.tensor_scalar(
            v_sb[:].rearrange("p (n d) -> p n d", d=128)[:, :, 1:64],
            xt_sb[:, 0:64 * 63].rearrange("p (n d) -> p n d", d=63),
            0.0, 0.0, mybir.AluOpType.mult, mybir.AluOpType.add)

        def qkv_step(tq):
            qk_block(0, tq)
            qk_block(2, tq)
            v_block_pair(4 * tq, 4 * tq + 1)
            v_block_pair(4 * tq + 2, 4 * tq + 3)
            qk_block(1, tq)
            qk_block(3, tq)

        # ---- attention ----
        # Diagonal-block column starts: m = kt - 4*tqb >= 0 means keys
        # overlap queries. Live range is [128*m, 512); S/exp/AV stream
        # [c0, 512) with c0 = 128*m capped at 256 so fp32r keeps 1 cy/row.
        # m==3 therefore includes 128 dead columns which mask2 zeroes.
        def attn_head(h, tqb):
            co_q = h // 2
            kp = 2 + h
            p0 = 64 * (h % 2)
            nkt = 4 * (tqb + 1)
            av = psum.tile([128, 512], F32, tag="av", bufs=2)
            for g in range(nkt // 2):
                s = psum.tile([128, 1024], F32, tag="s", bufs=2)
                e = work.tile([128, 1024], F32R, tag="e", bufs=2)
                ms = [2 * g + j - 4 * tqb for j in range(2)]
                c0s = [min(max(m, 0) * 128, 256) for m in ms]
                for j in range(2):
                    kt = 2 * g + j
                    c0 = c0s[j]
                    nc.tensor.matmul(
                        s[:, j * 512 + c0: (j + 1) * 512],
                        qk_sb[:, kp * T + kt * 128: kp * T + (kt + 1) * 128],
                        qk_sb[:, co_q * T + tqb * 512 + c0: co_q * T + (tqb + 1) * 512],
                        start=True, stop=True)
                # exp via ScalarE (no max subtraction: |scores| <= ~8 here,
                # exp is safe in fp32). Restricted to live columns.
                if ms[1] < 1:
                    nc.scalar.activation(e[:], s[:], AF.Exp, scale=SCALE)
                elif ms[0] == 0:  # (m0, m1) pair: [0:512] and [128:512]
                    nc.scalar.activation(e[:, 0:512], s[:, 0:512], AF.Exp, scale=SCALE)
                    nc.scalar.activation(e[:, 512 + 128:1024], s[:, 512 + 128:1024],
                                         AF.Exp, scale=SCALE)
                else:  # (m2, m3) pair: both [256:512] — one strided act
                    ev = e[:].rearrange("p (j q) -> p j q", q=512)[:, :, 256:512]
                    sv = s[:].rearrange("p (j q) -> p j q", q=512)[:, :, 256:512]
                    nc.scalar.activation(ev, sv, AF.Exp, scale=SCALE)
                for j in range(2):
                    kt = 2 * g + j
                    m = ms[j]
                    c0 = c0s[j]
                    if m >= 0:  # diagonal block: triangular mask
                        if m == 3:
                            nc.vector.tensor_mul(e[:, j * 512 + 256: j * 512 + 512],
                                                 e[:, j * 512 + 256: j * 512 + 512],
                                                 mask2_sb)
                        else:
                            mc = 128 * m
                            nc.vector.tensor_mul(e[:, j * 512 + mc: j * 512 + mc + 128],
                                                 e[:, j * 512 + mc: j * 512 + mc + 128],
                                                 tri_sb)
                    nc.tensor.matmul(
                        av[:, c0:512],
                        v_sb[:, kt * 512 + h * 128: kt * 512 + (h + 1) * 128],
                        e[:, j * 512 + c0: (j + 1) * 512],
                        start=(kt == 0), stop=(kt == nkt - 1))
            # normalize: attn[:, cols] = av[64:128] * (1/av[0]).
            # av[0] is the denominator row at PSUM partition 0 (the custom
            # DVE reciprocal misreads nonzero partition offsets).
            recipf = work.tile([1, 512], F32, tag="recipf", bufs=1)
            nc.vector.reciprocal_approx_fast(recipf[:], av[0:1, :])
            bcs = work.tile([64, 512], F32, tag="bcs")
            nc.gpsimd.partition_broadcast(bcs[:], recipf[:])
            nc.vector.tensor_mul(
                attn_sb[p0:p0 + 64, (h // 2) * T + tqb * 512: (h // 2) * T + (tqb + 1) * 512],
                av[64:128, :], bcs[:])

        def proj_block(t16):
            ysb = work.tile([128, C], BF16, tag="y")
            for n in range(2):
                p = psum.tile([128, 512], F32, tag="mm", bufs=2)
                for kc in range(2):
                    nc.tensor.matmul(p[:],
                                     attn_sb[:, kc * T + t16 * 128: kc * T + (t16 + 1) * 128],
                                     wp_sb[:, kc * C + n * 512: kc * C + (n + 1) * 512],
                                     start=(kc == 0), stop=(kc == 1))
                # PSUM->SBUF drain split between ScalarE and DVE (GpSimd
                # cannot access PSUM); b_proj is folded in on the host.
                # Each 512-half DMAs out as soon as its copy lands.
                if n == 0:
                    nc.scalar.copy(ysb[:, n * 512:(n + 1) * 512], p[:])
                else:
                    nc.vector.tensor_copy(ysb[:, n * 512:(n + 1) * 512], p[:])
            nc.sync.dma_start(y[t16 * 128:(t16 + 1) * 128, :], ysb[:])

        # Software pipeline: QKV for tq+1 is spliced between attention heads
        # of tq so the PE has dense independent work while ScalarE runs exp.
        qkv_step(0)
        for tqb in range(NTQ):
            nxt = tqb + 1
            prv = tqb - 1
            attn_head(0, tqb)
            if nxt < NTQ:
                qk_block(0, nxt)
                qk_block(2, nxt)
            if prv >= 0:
                proj_block(4 * prv + 0)
                proj_block(4 * prv + 1)
            attn_head(1, tqb)
            if nxt < NTQ:
                v_block_pair(4 * nxt, 4 * nxt + 1)
                v_block_pair(4 * nxt + 2, 4 * nxt + 3)
            attn_head(2, tqb)
            if nxt < NTQ:
                qk_block(1, nxt)
            if prv >= 0:
                proj_block(4 * prv + 2)
            attn_head(3, tqb)
            if nxt < NTQ:
                qk_block(3, nxt)
            if prv >= 0:
                proj_block(4 * prv + 3)
        for t16 in range(4 * 3, 4 * 4):
            proj_block(t16)

    nc.compile()
    return nc


def _get_nc():
    global _cached_nc
    if _cached_nc is None:
        _cached_nc = _build()
    return _cached_nc


def make_in_maps(x, w_attn, b_attn, w_proj, b_proj):
    x = np.asarray(x, np.float32)
    w_attn = np.asarray(w_attn, np.float32)
    b_attn = np.asarray(b_attn, np.float32)
    w_proj = np.asarray(w_proj, np.float32)
    bf = ml_dtypes.bfloat16
    tri = np.triu(np.ones((128, 128), np.float32))
    mask2 = np.concatenate([np.zeros((128, 128), np.float32), tri], axis=1)
    kmask = np.concatenate([np.repeat([[1.0], [0.0]], 64, axis=0),
                            np.repeat([[0.0], [1.0]], 64, axis=0)],
                           axis=1).astype(np.float32)
    in_maps = []
    for core in range(N_CORES):
        b, hg = core // 4, core % 4
        cs = slice(hg * 256, (hg + 1) * 256)
        wqk_c = np.ascontiguousarray(
            np.concatenate([w_attn[:, cs], w_attn[:, 1024 + hg * 256:1024 + (hg + 1) * 256]],
                           axis=1)).astype(bf)
        bqk_vec = np.concatenate([b_attn[cs], b_attn[1024 + hg * 256:1024 + (hg + 1) * 256]])
        consts = np.zeros((128, NCONST), np.float32)
        consts[:, CO_BQK:CO_BQK + 4] = bqk_vec.reshape(4, 128).T
        consts[:, CO_KM:CO_KM + 2] = kmask
        consts[:, CO_BV:CO_BV + CV] = np.broadcast_to(
            b_attn[2048 + hg * 256:2048 + (hg + 1) * 256], (128, 256))
        consts[:, CO_TRI:CO_TRI + 128] = tri
        consts[:, CO_M2:CO_M2 + 256] = mask2
        # chunk-major xt: xtc[p, q*4096 + k*512 + t] = x[b, q*512 + t, k*128 + p]
        xtc = np.ascontiguousarray(
            x[b].reshape(4, 512, 8, 128).transpose(3, 0, 2, 1).reshape(128, KT * T))
        wv_c = w_attn[:, 2048 + hg * 256:2048 + (hg + 1) * 256]
        in_maps.append({
            "xt": xtc.astype(bf),
            "wqk": np.ascontiguousarray(
                wqk_c.reshape(8, 128, CQK).transpose(1, 0, 2).reshape(128, KT * CQK)),
            "wv": np.ascontiguousarray(
                wv_c.reshape(8, 128, CV).transpose(1, 0, 2).reshape(128, KT * CV)).astype(bf),
            "wp": np.ascontiguousarray(
                w_proj[cs, :].reshape(2, 128, C).transpose(1, 0, 2).reshape(128, 2 * C)).astype(bf),
            "consts": consts,
        })
    return in_maps


def kernel(x, w_attn, b_attn, w_proj, b_proj):
    in_maps = make_in_maps(x, w_attn, b_attn, w_proj, b_proj)
    nc = _get_nc()
    res = run_bass_kernel_spmd(nc, in_maps, core_ids=list(range(N_CORES)))
    b_proj = np.asarray(b_proj, np.float32)
    y = np.zeros((B, T, C), np.float32)
    for core in range(N_CORES):
        y[core // 4] += np.asarray(res.results[core]["y"], dtype=np.float32)
    y += b_proj
    return y
